# revision 20
# baseline (speedup 1.0000x reference)
"""Trainium2 Bass kernel for nn_Net_18021682774696 (MTGNN-style GNN).

Strategy: data-parallel over batch B=8 -> 1 batch per NeuronCore.
All three adjacency matrices are SBUF-resident in fp8e4 (pre-scaled by
2^12 so entries land in e4m3 normal range).  Propagation runs as fp8
col-strip matmuls (three branches at tile_position (0,32j) share one
PSUM tile and pipeline to ~84ns per 512-col matmul); layer-0 hop 1 uses
branch-major DoubleRow chains instead so branch j can start as soon as
its adjacency lands.  The alpha*x mixprop term is folded into the mlp
x-weight on the host (ga += alpha*(g1s+g2s)), so stacks hold raw
propagation psums and the hop-2 stationary picks up alpha*x during the
psum->fp8 transpose copy.  Channel-mix convs, TCN, layernorm and the
skip/end head run in bf16; skip/end projections are pre-collapsed on
the host (endW @ skipW_i).
"""
import sys
import os

sys.path.insert(0, '/opt/trn_rl_repo')

import numpy as np
import ml_dtypes

# ----------------------------------------------------------------------------
# Patches: this container's walrus accepts only ONE sem-wait per instruction.
# Split multi-wait instructions (tile attaches one wait per processor).
# ----------------------------------------------------------------------------
import concourse.bass as bass
import concourse.mybir as mybir
import concourse.tile as tile
from concourse.vector_clock import ScopedClock
from concourse.bass_utils import run_bass_kernel_spmd


def _drain_and_barrier_split(self, tick_clock, wait_clock):
    nc = self.nc
    drain_inst = nc.sync.drain()
    wait_clock.add_sem_waits(
        drain_inst.ins, ScopedClock({None: tick_clock.global_clock})
    )
    waits = list(drain_inst.ins.sync_info.on_wait)
    if len(waits) > 1:
        si = drain_inst.ins.sync_info
        si.on_wait = [waits[0]]
        drain_inst.ins.sync_info = si
        for w in waits[1:]:
            d2 = nc.sync.drain()
            d2.ins.sync_info = mybir.SyncInfo(on_wait=[w], on_update=[])

    nc.all_engine_barrier()
    assert self.sems is not None
    popped = nc._tile_sem_poison_stack.pop()
    assert popped is self._sem_poison
    nc.clear_and_free_semaphores(list(self.sems.allocated().values()))
    nc.all_engine_barrier()


tile.TileContext._drain_and_barrier = _drain_and_barrier_split

_orig_postorder = tile.postorder_instruction_blocks
_split_counter = [0]


def _split_multi_waits(ordered, start_bb_name, postordered_blocks):
    for bb_name, insts in ordered.items():
        new_list = []
        for inst in insts:
            si = getattr(inst, 'sync_info', None)
            waits = list(si.on_wait) if si is not None else []
            if len(waits) > 1:
                for w in waits[:-1]:
                    _split_counter[0] += 1
                    nop = mybir.InstNoOp(
                        name=f"I-waitsplit-{_split_counter[0]}", ins=[], outs=[])
                    nop.engine = inst.engine
                    nop.sync_info = mybir.SyncInfo(on_wait=[w], on_update=[])
                    new_list.append(nop)
                si.on_wait = [waits[-1]]
                inst.sync_info = si
            new_list.append(inst)
        ordered[bb_name] = new_list
    return _orig_postorder(ordered, start_bb_name, postordered_blocks)


tile.postorder_instruction_blocks = _split_multi_waits

# ----------------------------------------------------------------------------
# Model constants (hardcoded from the problem spec)
# ----------------------------------------------------------------------------
B, N, C, H, S, T = 8, 2048, 32, 128, 256, 12
LAYERS, NUM_TCN, GDEP = 3, 2, 2
ALPHA, EPS = 0.05, 1e-5
BF16 = mybir.dt.bfloat16
F32 = mybir.dt.float32
FP8 = mybir.dt.float8e4
DR = mybir.MatmulPerfMode.DoubleRow
KCH = N // 128          # 16 contraction chunks
KPAIR = KCH // 2        # 8 DoubleRow steps
NCH = N // 512          # 4 psum n-chunks
NBF = ml_dtypes.bfloat16
E4 = ml_dtypes.float8_e4m3
SA = float(2 ** 12)     # adjacency fp8 pre-scale
SH = 8.0                # h1 stationary fp8 pre-scale
SAH = SA * SH

_prog_cache = {}


def _build(has_affine: bool):
    nc = bass.Bass(trn_type="TRN2", name="gnn_mp")
    ts, AF, ALU = bass.ts, mybir.ActivationFunctionType, mybir.AluOpType

    # ---- DRAM I/O ----
    adj = [nc.dram_tensor(f"adj{j}", [N, N], FP8, kind="ExternalInput")
           for j in range(3)]                      # 0 static, 1 dy, 2 dyT
    x0_d = nc.dram_tensor("x0", [C, N], BF16, kind="ExternalInput")
    x0T_d = nc.dram_tensor("x0T", [128, KCH * C], FP8, kind="ExternalInput")
    embs_d = nc.dram_tensor("embs", [96, N], BF16, kind="ExternalInput")
    id3_d = nc.dram_tensor("id3", [96, 32], BF16, kind="ExternalInput")
    id3s_d = nc.dram_tensor("id3s", [96, 32], BF16, kind="ExternalInput")
    tcnW_d = nc.dram_tensor("tcnW", [H, LAYERS * 4 * H], BF16, kind="ExternalInput")
    tcnB_d = nc.dram_tensor("tcnB", [H, LAYERS * 4], F32, kind="ExternalInput")
    g1s_d = nc.dram_tensor("g1s", [96, LAYERS * 32], BF16, kind="ExternalInput")
    g2s_d = nc.dram_tensor("g2s", [97, LAYERS * 32], BF16, kind="ExternalInput")
    ga_d = nc.dram_tensor("ga", [32, LAYERS * 32], BF16, kind="ExternalInput")
    ew_d = nc.dram_tensor("ew", [H, LAYERS * T], BF16, kind="ExternalInput")
    ewe_d = nc.dram_tensor("ewe", [C, T], BF16, kind="ExternalInput")
    cb_d = nc.dram_tensor("cb", [T, 1], F32, kind="ExternalInput")
    if has_affine:
        nw_d = nc.dram_tensor("nw", [C, LAYERS * N], BF16, kind="ExternalInput")
        nb_d = nc.dram_tensor("nb", [C, LAYERS * N], BF16, kind="ExternalInput")
    out_d = nc.dram_tensor("out", [T, N], F32, kind="ExternalOutput")

    with tile.TileContext(nc) as tc:
        with (
            tc.tile_pool(name="adjp", bufs=1) as adjp,
            tc.tile_pool(name="cst", bufs=1) as cst,
            tc.tile_pool(name="wk", bufs=1) as wk,
            tc.tile_pool(name="sc", bufs=2) as scp,
            tc.tile_pool(name="hc", bufs=1) as hcp,
            tc.tile_pool(name="ps", bufs=8, space="PSUM") as ps,
        ):
            # ---- persistent SBUF ----
            a_t = [adjp.tile([128, KCH, N], FP8, name=f"a{j}") for j in range(3)]
            hid = cst.tile([H, N], BF16, name="hid")        # x rows 0:32, embs 32:128
            id3 = cst.tile([96, 32], BF16, name="id3")
            id3s = cst.tile([96, 32], BF16, name="id3s")
            tcnW = cst.tile([H, LAYERS * 4 * H], BF16, name="tcnW")
            tcnB = cst.tile([H, LAYERS * 4], F32, name="tcnB")
            g1s = cst.tile([96, LAYERS * 32], BF16, name="g1s")
            g2s = cst.tile([97, LAYERS * 32], BF16, name="g2s")
            ga = cst.tile([32, LAYERS * 32], BF16, name="ga")
            ew = cst.tile([H, LAYERS * T], BF16, name="ew")
            ewe = cst.tile([C, T], BF16, name="ewe")
            cb = cst.tile([T, 1], F32, name="cb")
            ones1 = cst.tile([1, 32], F32, name="ones1")
            ones32f = cst.tile([32, 1], F32, name="ones32f")
            epsCN2 = cst.tile([1, 1], F32, name="epsCN2")
            if has_affine:
                nw = cst.tile([C, LAYERS * N], BF16, name="nw")
                nb = cst.tile([C, LAYERS * N], BF16, name="nb")

            x_wc = wk.tile([128, KCH, 32], FP8, name="x_wc")
            h1_wc = [wk.tile([128, KCH, 32], FP8, name=f"h1wc{j}") for j in range(3)]
            stack1 = wk.tile([96, N], BF16, name="stack1")
            stack2 = wk.tile([97, N], BF16, name="stack2")
            out_acc = wk.tile([T, N], F32, name="out_acc")
            sums = wk.tile([32, 2 * NCH], F32, name="sums")
            stat = wk.tile([1, 4], F32, name="stat")
            bc = wk.tile([32, 2], F32, name="bc")
            eps_t = wk.tile([1, 1], F32, name="eps_t")

            # ---- loads: adjacencies strict-priority a0->a1->a2 on gpsimd
            # (a0's first chunks split finer for latency); TCN-critical
            # constants on the scalar issuer, the rest on sync.
            for k in range(4):
                for h in range(2):
                    nc.gpsimd.dma_start(
                        a_t[0][:, k, bass.ds(h * 1024, 1024)],
                        adj[0][ts(k, 128), bass.ds(h * 1024, 1024)])
            for k in range(4, KCH):
                nc.gpsimd.dma_start(a_t[0][:, k, :], adj[0][ts(k, 128), :])
            nc.sync.dma_start(x_wc[:].rearrange("p k c -> p (k c)"), x0T_d[:])
            for q in range(2):
                nc.scalar.dma_start(hid[0:C, ts(q, N // 2)], x0_d[:, ts(q, N // 2)])
            for q in range(4):
                nc.scalar.dma_start(hid[C:H, ts(q, N // 4)],
                                    embs_d[:, ts(q, N // 4)])
            nc.scalar.dma_start(tcnB[:], tcnB_d[:])
            for q in range(4):
                w = LAYERS * H
                nc.sync.dma_start(tcnW[:, ts(q, w)], tcnW_d[:, ts(q, w)])
            nc.sync.dma_start(id3[:], id3_d[:])
            nc.sync.dma_start(id3s[:], id3s_d[:])
            nc.sync.dma_start(g1s[:], g1s_d[:])
            nc.sync.dma_start(g2s[:], g2s_d[:])
            nc.sync.dma_start(ga[:], ga_d[:])
            nc.sync.dma_start(ew[:], ew_d[:])
            nc.sync.dma_start(ewe[:], ewe_d[:])
            nc.sync.dma_start(cb[:], cb_d[:])
            if has_affine:
                nc.sync.dma_start(nw[:], nw_d[:])
                nc.sync.dma_start(nb[:], nb_d[:])
            nc.vector.memset(ones1[:], 1.0)
            nc.vector.memset(eps_t[:], EPS)
            nc.vector.memset(ones32f[:], 1.0)
            nc.vector.memset(stack2[96:97, :], 1.0)
            nc.vector.memset(epsCN2[:], EPS * (C * N) ** 2)
            nc.vector.memset(out_acc[:], 0.0)
            for k in range(KCH):
                nc.gpsimd.dma_start(a_t[1][:, k, :], adj[1][ts(k, 128), :])
            for k in range(KCH):
                nc.gpsimd.dma_start(a_t[2][:, k, :], adj[2][ts(k, 128), :])

            def prop_hop_strips(stats, stack, pre_hook=None, post_hook=None):
                """One propagation hop: 3 branches as col strips sharing one
                PSUM tile, k-interleaved so the strips pipeline on the PE.
                post_hook(q) is issued AFTER chunk q+1's matmuls so its PE
                consumers never stall on chunk q's DVE copy."""
                for n in range(NCH):
                    if pre_hook:
                        pre_hook(n)
                    pt = ps.tile([128, 512], F32, tag="ps", name=f"pt{n}")
                    for ki in range(KCH):
                        for j in range(3):
                            nc.tensor.matmul(
                                pt[32 * j:32 * j + 32, :], stats[j][:, ki, :],
                                a_t[j][:, ki, ts(n, 512)],
                                start=(ki == 0), stop=(ki == KCH - 1),
                                tile_position=(0, 32 * j))
                    nc.vector.tensor_copy(stack[0:96, ts(n, 512)], pt[0:96, :])
                    if post_hook and n >= 1:
                        post_hook(n - 1)
                if post_hook:
                    post_hook(NCH - 1)

            def transpose_pair(src_ap_fn, dst, tpos_row, q, ident, ax=None):
                """dst[:, 8q:8q+8, :] <- transpose of src cols 1024q..1024q+1024
                (two 512-chunks in one psum + one DVE op).  With ax=(x_tile,
                scale): dst = scale*x_tile_slice + transpose."""
                tp = ps.tile([128, 256], F32, tag="ps", name="tp")
                for r in range(8):
                    k = 8 * q + r
                    nc.tensor.matmul(
                        tp[:, 32 * r:32 * r + 32],
                        src_ap_fn(k), ident[tpos_row:tpos_row + 32, :],
                        start=True, stop=True, tile_position=(tpos_row, 0))
                dst_ap = dst[:, 8 * q:8 * q + 8, :].rearrange("p a b -> p (a b)")
                if ax is None:
                    nc.vector.tensor_copy(dst_ap, tp[:])
                else:
                    x_tile, scale = ax
                    nc.vector.scalar_tensor_tensor(
                        out=dst_ap,
                        in0=x_tile[:, 8 * q:8 * q + 8, :].rearrange("p a b -> p (a b)"),
                        scalar=scale, in1=tp[:], op0=ALU.mult, op1=ALU.add)

            # ================= layers =================
            for i in range(LAYERS):
                wf1 = tcnW[:, ts(i * 4 + 0, H)]
                wg1 = tcnW[:, ts(i * 4 + 1, H)]
                wf2 = tcnW[:, ts(i * 4 + 2, H)]
                wg2 = tcnW[:, ts(i * 4 + 3, H)]
                bf1 = tcnB[:, i * 4 + 0:i * 4 + 1]
                bg1 = tcnB[:, i * 4 + 1:i * 4 + 2]
                bf2 = tcnB[:, i * 4 + 2:i * 4 + 3]
                bg2 = tcnB[:, i * 4 + 3:i * 4 + 4]

                htc = [None] * NCH
                h2cs = [None] * NCH

                def tcn_wave(u, chunks):
                    wf, wg = (wf1, wg1) if u == 0 else (wf2, wg2)
                    bf, bg = (bf1, bg1) if u == 0 else (bf2, bg2)
                    for n in chunks:
                        src_ap = hid[:, ts(n, 512)] if u == 0 else htc[n][:]
                        pf = ps.tile([H, 512], F32, tag="ps", name="pf")
                        pg = ps.tile([H, 512], F32, tag="ps", name="pg")
                        nc.tensor.matmul(pf[:], wf, src_ap, start=True, stop=True)
                        nc.tensor.matmul(pg[:], wg, src_ap, start=True, stop=True)
                        tf = scp.tile([H, 512], BF16, tag="tf", name="tf")
                        tg = scp.tile([H, 512], BF16, tag="tg", name="tg")
                        nc.scalar.activation(tf[:], pf[:], AF.Tanh, bias=bf)
                        nc.scalar.activation(tg[:], pg[:], AF.Sigmoid, bias=bg)
                        ht = hcp.tile([H, 512], BF16, tag=f"htc{n}", name="ht")
                        nc.vector.tensor_mul(ht[:], tf[:], tg[:])
                        if u == 0:
                            htc[n] = ht
                        else:
                            h2cs[n] = ht

                def transposes1(j, q):
                    # h1_wc[j] = SH*h1^T = alpha*SH*x^T + (SH/SA)*stack1^T
                    transpose_pair(
                        lambda k: stack1[32 * j:32 * j + 32, ts(k, 128)],
                        h1_wc[j], 32 * j, q, id3s, ax=(x_wc, ALPHA * SH))

                def transposes1_all(q):
                    # all 3 branches interleaved across PE row groups so the
                    # transpose LDWEIGHTS overlap in-flight matmuls
                    tps = [ps.tile([128, 256], F32, tag="ps", name=f"tq{j}")
                           for j in range(3)]
                    for r in range(8):
                        k = 8 * q + r
                        for j in range(3):
                            nc.tensor.matmul(
                                tps[j][:, 32 * r:32 * r + 32],
                                stack1[32 * j:32 * j + 32, ts(k, 128)],
                                id3s[32 * j:32 * j + 32, :],
                                start=True, stop=True,
                                tile_position=(32 * j, 0))
                    for j in range(3):
                        nc.vector.scalar_tensor_tensor(
                            out=h1_wc[j][:, 8 * q:8 * q + 8, :]
                            .rearrange("p a b -> p (a b)"),
                            in0=x_wc[:, 8 * q:8 * q + 8, :]
                            .rearrange("p a b -> p (a b)"),
                            scalar=ALPHA * SH, in1=tps[j][:],
                            op0=ALU.mult, op1=ALU.add)

                # ---- TCN unit 1 + propagation hop 1 ----
                tcn_wave(0, [0, 1])
                if i == 0:
                    # branch-major DoubleRow: branch j starts once adj_j lands
                    for j in range(3):
                        for n in range(NCH):
                            if j == 0 and n == 1:
                                tcn_wave(0, [2, 3])
                            pt = ps.tile([32, 512], F32, tag="ps", name=f"pp{j}{n}")
                            for ki in range(KPAIR):
                                nc.tensor.matmul(
                                    pt[:], x_wc[:, 2 * ki:2 * ki + 2, :],
                                    a_t[j][:, 2 * ki:2 * ki + 2, ts(n, 512)],
                                    start=(ki == 0), stop=(ki == KPAIR - 1),
                                    perf_mode=DR)
                            nc.vector.tensor_copy(
                                stack1[32 * j:32 * j + 32, ts(n, 512)], pt[:])
                        if j >= 1:
                            # transpose branch j-1 while branch j streams
                            for q in range(2):
                                transposes1(j - 1, q)
                    for q in range(2):
                        transposes1(2, q)
                else:
                    def pre1(n):
                        if n == 1:
                            tcn_wave(0, [2, 3])

                    def post1(q):
                        if q == 1:
                            transposes1_all(0)
                        elif q == 3:
                            transposes1_all(1)
                    prop_hop_strips([x_wc] * 3, stack1,
                                    pre_hook=pre1, post_hook=post1)

                # ---- TCN unit 2 + propagation hop 2 + mlp/stats ----
                tcn_wave(1, [0, 1])
                pm = [None] * NCH

                def mlp_block(n):
                    pmn = ps.tile([32, 512], F32, tag="ps", name=f"pm{n}")
                    pm[n] = pmn
                    nc.tensor.matmul(pmn[:], ga[:, ts(i, 32)],
                                     hid[0:32, ts(n, 512)], start=True, stop=False)
                    nc.tensor.matmul(pmn[:], g1s[:, ts(i, 32)],
                                     stack1[:, ts(n, 512)], start=False, stop=False)
                    nc.tensor.matmul(pmn[:], g2s[:, ts(i, 32)],
                                     stack2[0:97, ts(n, 512)], start=False, stop=True)
                    pk = ps.tile([T, 512], F32, tag="ps", name="pk")
                    nc.tensor.matmul(pk[:], ew[:, ts(i, T)], h2cs[n][:],
                                     start=True, stop=True)
                    nc.vector.tensor_add(out_acc[:, ts(n, 512)],
                                         out_acc[:, ts(n, 512)], pk[:])
                    nc.vector.tensor_reduce(sums[:, n:n + 1], pmn[:],
                                            mybir.AxisListType.X, ALU.add)
                    sq = scp.tile([32, 512], BF16, tag="tmp", name="sq")
                    nc.scalar.activation(sq[:], pmn[:], AF.Square,
                                         accum_out=sums[:, NCH + n:NCH + n + 1])

                def pre2(n):
                    if n == 1:
                        tcn_wave(1, [2, 3])
                        # dummy sqrt: swap the ACT table mid-hop, off the
                        # critical path (Square/Relu live in every table)
                        nc.scalar.activation(stat[:, 3:4], stat[:, 3:4], AF.Sqrt,
                                             bias=eps_t[:])
                prop_hop_strips(h1_wc, stack2, pre_hook=pre2, post_hook=mlp_block)

                # ---- layernorm scalar chain (all tiny psums share one bank) ----
                lnp = ps.tile([32, 16], F32, tag="ps", name="lnp")
                nc.tensor.matmul(lnp[0:1, 0:2 * NCH], ones32f[:], sums[:],
                                 start=True, stop=True)
                nc.vector.tensor_reduce(stat[:, 0:1], lnp[0:1, 0:NCH],
                                        mybir.AxisListType.X, ALU.add)
                nc.vector.tensor_reduce(stat[:, 1:2], lnp[0:1, NCH:2 * NCH],
                                        mybir.AxisListType.X, ALU.add)
                nc.vector.tensor_mul(stat[:, 2:3], stat[:, 0:1], stat[:, 0:1])
                nc.tensor.matmul(lnp[0:1, 8:9], eps_t[:], stat[:, 2:3],
                                 start=True, stop=True)
                nc.vector.scalar_tensor_tensor(
                    out=stat[:, 1:2], in0=stat[:, 1:2], scalar=float(C * N),
                    in1=stat[:, 2:3], op0=ALU.mult, op1=ALU.subtract)
                nc.scalar.activation(stat[:, 1:2], stat[:, 1:2], AF.Sqrt,
                                     bias=epsCN2[:])
                # dummy sigmoid: preload the sigmoid table (the only one with
                # sigmoid+tanh+relu+square) during the bc chain
                nc.scalar.activation(stat[:, 3:4], stat[:, 3:4], AF.Sigmoid)
                nc.tensor.matmul(lnp[0:1, 9:10], eps_t[:], stat[:, 1:2],
                                 start=True, stop=True)
                nc.vector.reciprocal(stat[:, 1:2], stat[:, 1:2])
                nc.vector.tensor_scalar_mul(stat[:, 2:3], stat[:, 1:2], float(C * N))
                nc.tensor.matmul(lnp[0:1, 10:11], eps_t[:], stat[:, 2:3],
                                 start=True, stop=True)
                nc.vector.scalar_tensor_tensor(
                    out=stat[:, 3:4], in0=stat[:, 0:1], scalar=-1.0,
                    in1=stat[:, 1:2], op0=ALU.mult, op1=ALU.mult)
                nc.tensor.matmul(lnp[:, 12:14], ones1[:], stat[:, 2:4],
                                 start=True, stop=True)
                nc.vector.tensor_copy(bc[:], lnp[:, 12:14])

                # ---- normalize + relu -> hid[0:32] (scalar engine), + x transpose
                for n in range(NCH):
                    if has_affine:
                        tmp = scp.tile([32, 512], BF16, tag="tmp", name="tmp")
                        nc.vector.tensor_scalar(
                            out=tmp[:], in0=pm[n][:],
                            scalar1=bc[:, 0:1], scalar2=bc[:, 1:2],
                            op0=ALU.mult, op1=ALU.add)
                        nc.vector.tensor_mul(tmp[:], tmp[:],
                                             nw[:, bass.ds(i * N + n * 512, 512)])
                        nc.vector.tensor_add(tmp[:], tmp[:],
                                             nb[:, bass.ds(i * N + n * 512, 512)])
                        nc.vector.tensor_scalar_max(hid[0:32, ts(n, 512)], tmp[:], 0.0)
                    else:
                        nc.scalar.activation(
                            hid[0:32, ts(n, 512)], pm[n][:], AF.Relu,
                            scale=bc[:, 0:1], bias=bc[:, 1:2])
                    if i < LAYERS - 1 and n % 2 == 1:
                        transpose_pair(lambda k: hid[0:32, ts(k, 128)],
                                       x_wc, 0, n // 2, id3)

            # ---- final head: out = out_acc + EWE @ x3 + cb ----
            for n in range(NCH):
                pk = ps.tile([T, 512], F32, tag="ps", name="pk")
                nc.tensor.matmul(pk[:], ewe[:], hid[0:32, ts(n, 512)],
                                 start=True, stop=True)
                nc.vector.scalar_tensor_tensor(
                    out=out_acc[:, ts(n, 512)], in0=pk[:], scalar=cb[:],
                    in1=out_acc[:, ts(n, 512)], op0=ALU.add, op1=ALU.add)
                nc.scalar.dma_start(out_d[:, ts(n, 512)], out_acc[:, ts(n, 512)])

    return nc


def _prep(inputs):
    """Host-side preprocessing -> per-core input maps."""
    f32 = np.float32
    x = inputs['x'].astype(f32).reshape(B, C, N)
    dy = inputs['dy_graph'].astype(f32)
    S_ = inputs['static_graph'].astype(f32)
    sp = inputs['spatial_emb'].astype(f32).reshape(B, 32, N)
    td = inputs['temporal_d_emb'].astype(f32).reshape(B, 32, N)
    tw = inputs['temporal_w_emb'].astype(f32).reshape(B, 32, N)

    sc = np.float32((1.0 - ALPHA) * SA)
    # static: rhs0[w,v] = (S^T + I)[w,v] * (1-a)*SA/r0[v],  r0 = S.sum(1)+1
    r0 = S_.sum(1) + 1.0
    adj0 = ((S_.T + np.eye(N, dtype=f32)) * (sc / r0)[None, :]).astype(E4)
    adj1 = np.empty((B, N, N), E4)
    adj2 = np.empty((B, N, N), E4)
    for b in range(B):
        d = dy[b]
        r1 = d.sum(1) + 1.0
        r2 = d.sum(0) + 1.0
        dT = np.ascontiguousarray(d.T)
        adj1[b] = ((dT + np.eye(N, dtype=f32)) * (sc / r1)[None, :]).astype(E4)
        adj2[b] = ((d + np.eye(N, dtype=f32)) * (sc / r2)[None, :]).astype(E4)

    id3 = np.zeros((96, 32), f32)
    for j in range(3):
        id3[32 * j:32 * j + 32] = np.eye(32)
    id3s = (id3 * (SH / SA)).astype(NBF)
    id3 = id3.astype(NBF)

    # TCN weights: lhsT = W^T laid out [cin, (layer,unit,fg)*cout]
    tcnW = np.zeros((H, LAYERS * 4 * H), f32)
    tcnB = np.zeros((H, LAYERS * 4), f32)
    for i in range(LAYERS):
        for u in range(NUM_TCN):
            tcnW[:, (i * 4 + 2 * u) * H:(i * 4 + 2 * u + 1) * H] = \
                inputs['enc_Wf'][i, u].astype(f32).T
            tcnW[:, (i * 4 + 2 * u + 1) * H:(i * 4 + 2 * u + 2) * H] = \
                inputs['enc_Wg'][i, u].astype(f32).T
            tcnB[:, i * 4 + 2 * u] = inputs['enc_bf'][i, u].astype(f32)
            tcnB[:, i * 4 + 2 * u + 1] = inputs['enc_bg'][i, u].astype(f32)

    gW = [inputs['g0_W'].astype(f32), inputs['g1_W'].astype(f32),
          inputs['g2_W'].astype(f32)]
    gB = [inputs['g0_b'].astype(f32), inputs['g1_b'].astype(f32),
          inputs['g2_b'].astype(f32)]
    g1s = np.zeros((96, LAYERS * 32), f32)
    g2s = np.zeros((97, LAYERS * 32), f32)
    ga = np.zeros((32, LAYERS * 32), f32)
    for i in range(LAYERS):
        for k in range(3):   # branch k: 0 static, 1 dy, 2 dyT
            w1 = gW[k][i][:, 32:64].T      # applied to h1_k
            w2 = gW[k][i][:, 64:96].T      # applied to h2_k
            g1s[32 * k:32 * k + 32, 32 * i:32 * i + 32] = w1 / SA
            g2s[32 * k:32 * k + 32, 32 * i:32 * i + 32] = w2 / SAH
            # alpha*x folding: h1 = a*x + P1, h2 = a*x + P2
            ga[:, 32 * i:32 * i + 32] += gW[k][i][:, 0:32].T + ALPHA * (w1 + w2)
            g2s[96, 32 * i:32 * i + 32] += gB[k][i]

    for i in range(LAYERS):
        ga[:, 32 * i:32 * i + 32] += np.eye(32, dtype=f32)
    endW = inputs['end_W'].astype(f32)
    ew = np.zeros((H, LAYERS * T), f32)
    for i in range(LAYERS):
        ew[:, i * T:(i + 1) * T] = (endW @ inputs['skip_W'][i].astype(f32)).T
    ewe = (endW @ inputs['skipE_W'].astype(f32)).T
    cb = (endW @ (inputs['skip_b'].astype(f32).sum(0)
                  + inputs['skipE_b'].astype(f32))
          + inputs['end_b'].astype(f32)).reshape(T, 1)

    nw = inputs['norm_w'].astype(f32).reshape(LAYERS, C, N)
    nbb = inputs['norm_b'].astype(f32).reshape(LAYERS, C, N)
    has_affine = not (np.all(nw == 1.0) and np.all(nbb == 0.0))

    shared = {
        "adj0": adj0, "id3": id3, "id3s": id3s,
        "tcnW": tcnW.astype(NBF), "tcnB": tcnB,
        "g1s": g1s.astype(NBF), "g2s": g2s.astype(NBF),
        "ga": ga.astype(NBF),
        "ew": ew.astype(NBF), "ewe": ewe.astype(NBF), "cb": cb,
        "embs": None,  # per-core below
    }
    if has_affine:
        shared["nw"] = np.concatenate([nw[i] for i in range(LAYERS)], 1).astype(NBF)
        shared["nb"] = np.concatenate([nbb[i] for i in range(LAYERS)], 1).astype(NBF)

    in_maps = []
    for b in range(B):
        m = dict(shared)
        m["embs"] = np.concatenate([sp[b], td[b], tw[b]], 0).astype(NBF)
        m["x0"] = x[b].astype(NBF)
        m["x0T"] = np.ascontiguousarray(
            x[b].T.reshape(KCH, 128, C).transpose(1, 0, 2).reshape(128, KCH * C)
        ).astype(E4)
        m["adj1"] = adj1[b]
        m["adj2"] = adj2[b]
        in_maps.append(m)
    return in_maps, has_affine


LAST_EXEC_NS = None


def _install_profile_hook():
    import types
    import antenv
    if 'antenv.axon_hooks' not in sys.modules:
        mod = types.ModuleType('antenv.axon_hooks')
        holder = {}
        mod.set_axon_ntff_profile_hook = lambda h: holder.__setitem__('h', h)
        mod.get_axon_ntff_profile_hook = lambda: holder.get('h')
        sys.modules['antenv.axon_hooks'] = mod
        antenv.axon_hooks = mod
        from trn_agent_boot.trn_boot import _ntff_profile_via_ctypes
        mod.set_axon_ntff_profile_hook(
            _ntff_profile_via_ctypes('/opt/axon/libaxon_pjrt.so'))
    import concourse.bass_utils as bu
    bu.upload_artifacts = lambda tmpdir: tmpdir


def kernel(**inputs):
    global LAST_EXEC_NS
    in_maps, has_affine = _prep(inputs)
    if has_affine not in _prog_cache:
        _prog_cache[has_affine] = _build(has_affine)
    nc = _prog_cache[has_affine]
    trace = bool(os.environ.get("KERNEL_TRACE"))
    if trace:
        _install_profile_hook()
    res = run_bass_kernel_spmd(nc, in_maps, core_ids=list(range(B)), trace=trace)
    LAST_EXEC_NS = res.exec_time_ns
    out = np.stack([res.results[b]["out"] for b in range(B)], 0)
    return out.reshape(B, T, N, 1).astype(np.float32)
